# revision 45
# baseline (speedup 1.0000x reference)
"""Trainium2 Bass kernel for nn_DecoderPolicyGradient (teacher-forced LSTM decoder).

Model: B=128, T=20, E=H=512, V=10000.
  xs[t] = features (t=0) | embed(captions[:, t-1])
  (h, c) = LSTM(xs[t], (h, c));  logits[t] = h @ W_lin.T + b_lin
  out = logits, time-major flattened: [T*B, V] fp32.

Sharding: pure data-parallel over batch, B/8 = 16 rows per NeuronCore, no
collectives. Per-core plan (everything "transposed": the 128-partition axis
carries hidden/gate dims and batch lives in the free dim):

  1. XgT[2048, 320] = W_ih @ xs.T + (b_ih + b_hh): one batched matmul over
     all 20 steps; the bias rides the psum->SBUF copy (alternating ACT
     bias-activation / DVE tensor_scalar_add).
  2. 20 serial LSTM steps at B=16, gatesT[2048, 16] = W_hh @ h + XgT[:, t].
     The XgT addend is folded into the gates PSUM by a tiny identity
     matmul per gate group (psum += I.T @ xgT), so ACT applies
     sigmoid/tanh directly FROM PSUM - no per-step DVE gate adds.
     h0 == c0 == 0 in this model, so step 0 skips all matmuls:
     gates_0 = XgT[:, 0] read straight from SBUF, c_1 = i*g.
  3. logits[320, 10000] = H @ W_lin.T: 3 row-chunks x 20 vocab-slices of
     512, one unit ~= 4 matmuls of N=512. Units are interleaved into the
     recurrence step tails (1/step from step 8) and the rest run densely
     after. PSUM rotates over 4 banks; copies go to DVE (ACT every 3rd
     unit in the dense phase).

HAM (PE clock gate) management: the PE defaults to 1.2GHz and only reaches
2.4GHz after ~3.4us of sustained activity; the baseline spent 58us of its
95us PE-busy time throttled. Dummy spin matmuls fill the two DMA-bound
holes (startup, and the phase-1 -> recurrence gap waiting on W_hh) to keep
the clock pinned at 2.4GHz.
"""

import sys

sys.path.insert(0, "/opt/trn_rl_repo")

from contextlib import ExitStack

import ml_dtypes
import numpy as np

import concourse.mybir as mybir
import concourse.tile as tile
from concourse import bacc
from concourse.bass_utils import run_bass_kernel_spmd

BF16 = mybir.dt.bfloat16
F32 = mybir.dt.float32
AF = mybir.ActivationFunctionType

B, T, E, H, V = 128, 20, 512, 512, 10000
NC = 8
BL = B // NC  # 16 batch rows per core
R = BL * T  # 320 output rows per core
KT = 4  # k-tiles of 128 over E/H
GT = 16  # m-tiles of 128 over 4H
VS = 512  # vocab n-slice width
M_CHUNKS = ((0, 128), (128, 128), (256, 64))  # logits m-chunks (start, rows)
N_SLICES = [(s, min(VS, V - s)) for s in range(0, V, VS)]

SPIN1 = 4  # high-duty warmup matmuls (N=448) riding the startup DMA wait
SPIN2 = 3  # bridge from phase-1A end to W_hh arrival
TAIL_SPIN = 2  # junk matmuls per spare early-step tail (h-dep pinned)
XG_A = 64  # Xg columns computed up-front (steps 0-3); rest rides step tails

_cache = {}


def _build_nc(use_blin, h0_zero):
    nc = bacc.Bacc("TRN2", target_bir_lowering=False, debug=False)

    ident_d = nc.dram_tensor("ident", [128, 128], BF16, kind="ExternalInput").ap()
    bsum_d = nc.dram_tensor("bsum", [128, GT], F32, kind="ExternalInput").ap()
    wihT_d = nc.dram_tensor("wihT", [128, KT, 4 * H], BF16, kind="ExternalInput").ap()
    xsT_d = nc.dram_tensor("xsT", [128, KT, R], BF16, kind="ExternalInput").ap()
    whhT_d = nc.dram_tensor("whhT", [128, KT, 4 * H], BF16, kind="ExternalInput").ap()
    wlinT_d = nc.dram_tensor("wlinT", [128, KT, V], BF16, kind="ExternalInput").ap()
    blin_d = nc.dram_tensor("blin", [1, V], BF16, kind="ExternalInput").ap()
    if not h0_zero:
        h0T_d = nc.dram_tensor("h0T", [128, KT, BL], BF16, kind="ExternalInput").ap()
        c0T_d = nc.dram_tensor("c0T", [128, KT, BL], F32, kind="ExternalInput").ap()
    out_d = nc.dram_tensor("out", [R, V], BF16, kind="ExternalOutput").ap()

    with tile.TileContext(nc) as tc, ExitStack() as ctx:
        # few pools: each pool seems to cost a multi-engine barrier round
        # in the prologue/teardown (~1-2us each)
        const = ctx.enter_context(tc.tile_pool(name="const", bufs=1))
        work = ctx.enter_context(tc.tile_pool(name="work", bufs=2))
        psum_g = ctx.enter_context(tc.tile_pool(name="psum_g", bufs=1, space="PSUM"))
        stage = work
        psum_l = psum_g

        # ---- persistent SBUF tensors
        ident = const.tile([128, 128], BF16)
        xsT = const.tile([128, KT, R], BF16)
        wihT = const.tile([128, KT, 4 * H], BF16)
        bsum = const.tile([128, GT], F32)
        whhT = const.tile([128, KT, 4 * H], BF16)
        blin = const.tile([1, V], BF16)
        ones = const.tile([1, 128], BF16)
        wlinT = const.tile([128, KT, V], BF16)
        xgT = const.tile([128, GT, R], BF16)
        hstore = const.tile([128, KT, R], BF16)
        spinw = const.tile([128, 32], BF16)
        spinx = const.tile([128, 448], BF16)
        junk = const.tile([128, 1], F32)
        junk2 = const.tile([128, 1], BF16)
        if not h0_zero:
            h0T = const.tile([128, KT, BL], BF16)
            c0T = const.tile([128, KT, BL], F32)

        # ---- input DMAs (SP HWDGE ring, FIFO). Order = priority.
        nc.sync.dma_start(ident[:], ident_d[:])
        nc.sync.dma_start(bsum[:], bsum_d[:])
        # Descriptor issue on the sync engine costs ~0.7-1.4us EACH, so
        # keep the count low and order by when the data is needed:
        # xsT-head (phase-1A) first, wihT, xsT-mid (phase-1B), whhT
        # (recurrence), xsT-tail (phase-1C in step tails).
        nc.sync.dma_start(xsT[:, :, 0:XG_A], xsT_d[:, :, 0:XG_A])
        for q in range(2):
            nc.sync.dma_start(
                wihT[:, :, q * 1024 : (q + 1) * 1024],
                wihT_d[:, :, q * 1024 : (q + 1) * 1024],
            )
        nc.sync.dma_start(xsT[:, :, XG_A:192], xsT_d[:, :, XG_A:192])
        for q in range(2):
            nc.sync.dma_start(
                whhT[:, :, q * 1024 : (q + 1) * 1024],
                whhT_d[:, :, q * 1024 : (q + 1) * 1024],
            )
        nc.sync.dma_start(xsT[:, :, 192:R], xsT_d[:, :, 192:R])
        if not h0_zero:
            nc.sync.dma_start(h0T[:], h0T_d[:])
            nc.sync.dma_start(c0T[:], c0T_d[:])
        if use_blin:
            nc.sync.dma_start(blin[:], blin_d[:])

        nc.gpsimd.memset(spinw[:], 1.0)
        nc.gpsimd.memset(spinx[:], 0.5)
        nc.gpsimd.memset(junk[:], 0.0)
        nc.gpsimd.memset(ones[:], 1.0)

        # big W_lin load: the 16 SDMA engines drain the ring roughly
        # concurrently, so FIFO order alone would let this 10 MB steal HBM
        # bandwidth from the critical startup loads. Gate each chunk behind
        # whhT's arrival via a tiny gpsimd write into the chunk's
        # destination region (WAW) that itself reads whhT (RAW).
        def load_wlin_chunk(q):
            s = q * (V // 8)
            # gate on the LAST whhT chunk (ring FIFO: done last)
            nc.gpsimd.tensor_copy(
                wlinT[0:1, 0:1, s : s + 1], whhT[0:1, 3:4, 4 * H - 1 : 4 * H]
            )
            nc.sync.dma_start(
                wlinT[:, :, s : s + V // 8], wlinT_d[:, :, s : s + V // 8]
            )

        for q in range(4):
            load_wlin_chunk(2 * q)
            load_wlin_chunk(2 * q + 1)

        # ---- ACT table preloads (sigmoid/tanh/identity) ride the DMA wait
        nc.scalar.activation(junk2[:], junk[:], AF.Sigmoid)
        nc.scalar.activation(junk2[:], junk[:], AF.Tanh)
        nc.scalar.activation(junk2[:], junk[:], AF.Identity)

        # ---- warmup spin: keep the PE busy (and the HAM clock ramping to
        # 2.4GHz) while the startup DMAs land. High-duty shape: tiny 32-col
        # LDWEIGHTS + long N=448 matmul, ~93% MAC occupancy.
        psp = psum_g.tile([32, 448], F32, tag="sp")
        for _ in range(SPIN1):
            nc.tensor.matmul(psp[:], spinw[:], spinx[:], start=True, stop=True)

        # ---- phase 1: XgT[2048, R] = W_ih @ xs.T + bsum, in column blocks.
        # Block A (cols 0:XG_A) runs up-front; blocks B/C ride the early
        # recurrence step tails as real fill work (their columns are only
        # needed from step 4 / step 12 on).
        xg_blk = [0]

        def emit_xg_block(lo, hi, ms, allow_act=True):
            w = hi - lo
            for m in ms:
                pxg = psum_l.tile([128, R], F32, tag=f"pl{xg_blk[0] % 3}",
                                  name=f"pxg{lo}_{m}")
                xg_blk[0] += 1
                for k in range(KT):
                    nc.tensor.matmul(
                        pxg[:, 0:w],
                        wihT[:, k, m * 128 : (m + 1) * 128],
                        xsT[:, k, lo:hi],
                        start=(k == 0),
                        stop=(k == KT - 1),
                    )
                if allow_act and m % 2 == 0:
                    nc.scalar.activation(
                        xgT[:, m, lo:hi], pxg[:, 0:w], AF.Identity,
                        bias=bsum[:, m : m + 1],
                    )
                else:
                    nc.vector.tensor_scalar_add(
                        xgT[:, m, lo:hi], pxg[:, 0:w], bsum[:, m : m + 1]
                    )

        emit_xg_block(0, XG_A, range(GT), allow_act=False)
        # phase-1B right behind: real work bridging toward whhT arrival
        emit_xg_block(XG_A, 192, range(GT), allow_act=False)

        # ---- logits unit emitter
        uidx = [0]

        def mm_group(pl_ap, ms, rows, s, w):
            for k in range(KT):
                nc.tensor.matmul(
                    pl_ap,
                    hstore[:, k, ms : ms + rows],
                    wlinT[:, k, s : s + w],
                    start=(k == 0),
                    stop=(k == KT - 1) and not use_blin,
                )
            if use_blin:
                nc.tensor.matmul(
                    pl_ap, ones[:, :rows], blin[:, s : s + w],
                    start=False, stop=True,
                )

        def emit_copy_dma(pl, rows_parts, dmas, use_act, act_dma=False):
            ot = stage.tile([128, VS], BF16, tag=f"o{uidx[0] % 3}", bufs=6)
            cp = nc.scalar.copy if use_act else nc.vector.tensor_copy
            cp(ot[:rows_parts, :], pl[:rows_parts, :])
            dma_eng = nc.scalar if act_dma else nc.sync
            for (ms, rows, s, w, p0) in dmas:
                dma_eng.dma_start(
                    out_d[ms : ms + rows, s : s + w], ot[p0 : p0 + rows, :w]
                )

        def emit_logits_unit(ci, ni, use_act=False, act_dma=False):
            ms, rows = M_CHUNKS[ci]
            pl = psum_l.tile([128, VS], F32, tag=f"pl{uidx[0] % 3}")
            if rows == 128:
                s, w = N_SLICES[ni]
                mm_group(pl[:rows, :w], ms, rows, s, w)
                emit_copy_dma(pl, 128, [(ms, rows, s, w, 0)], use_act, act_dma)
            else:
                # ci == 2: ni indexes a PAIR of n-slices packed by row-half
                pair = N_SLICES[2 * ni : 2 * ni + 2]
                dmas = []
                for half, (s, w) in enumerate(pair):
                    mm_group(pl[64 * half : 64 * half + rows, :w], ms, rows, s, w)
                    dmas.append((ms, rows, s, w, 64 * half))
                emit_copy_dma(pl, 64 * len(pair), dmas, use_act, act_dma)
            uidx[0] += 1

        # ---- step 0 cell (h0 == c0 == 0): gates = XgT[:, :, 0:16] directly
        def emit_step0_zero():
            act_if = work.tile([128, 8, BL], BF16, tag="actif")
            act_g = work.tile([128, 4, BL], BF16, tag="actg")
            act_o = work.tile([128, 4, BL], BF16, tag="acto")
            nc.scalar.activation(act_if[:], xgT[:, 0:8, 0:BL], AF.Sigmoid)
            nc.scalar.activation(act_g[:], xgT[:, 8:12, 0:BL], AF.Tanh)
            nc.scalar.activation(act_o[:], xgT[:, 12:16, 0:BL], AF.Sigmoid)
            c_new = work.tile([128, 4, BL], F32, tag="c")
            nc.vector.tensor_mul(c_new[:], act_if[:, 0:4, :], act_g[:])
            tc_b = work.tile([128, 4, BL], BF16, tag="tanhc")
            nc.scalar.activation(tc_b[:], c_new[:], AF.Tanh)
            h_new = hstore[:, :, 0:BL]
            nc.vector.tensor_mul(h_new, act_o[:], tc_b[:])
            return h_new, c_new

        def emit_step(t, hT_prev, cT_prev):
            lo = t * BL
            # two psum tiles (separate banks) so sigmoid(i,f) only waits on
            # the first half of the matmul burst, not the whole thing
            pg_if = psum_g.tile([128, 8, BL], F32, tag=f"pa{t % 2}")
            pg_og = psum_g.tile([128, 8, BL], F32, tag=f"pb{t % 2}")
            for g in range(4):
                pg = pg_if if g < 2 else pg_og
                gl = (g % 2) * 4
                # seed the group's psum with XgT via an identity matmul;
                # the W_hh matmuls then accumulate on top of it.
                nc.tensor.matmul(
                    pg[:, gl : gl + 4, :],
                    ident[:],
                    xgT[:, 4 * g : 4 * g + 4, lo : lo + BL],
                    start=True,
                    stop=False,
                    skip_group_check=True,
                )
                for mi in range(4):
                    m = g * 4 + mi
                    for k in range(KT):
                        nc.tensor.matmul(
                            pg[:, gl + mi, :],
                            whhT[:, k, m * 128 : (m + 1) * 128],
                            hT_prev[:, k, :],
                            start=False,
                            stop=(k == KT - 1),
                            skip_group_check=True,
                        )
            act_if = work.tile([128, 8, BL], BF16, tag="actif")
            act_g = work.tile([128, 4, BL], BF16, tag="actg")
            act_o = work.tile([128, 4, BL], BF16, tag="acto")
            nc.scalar.activation(act_if[:], pg_if[:], AF.Sigmoid)
            nc.scalar.activation(act_g[:], pg_og[:, 0:4, :], AF.Tanh)
            ig = work.tile([128, 4, BL], F32, tag="ig")
            fc = work.tile([128, 4, BL], F32, tag="fc")
            nc.vector.tensor_mul(ig[:], act_if[:, 0:4, :], act_g[:])
            nc.vector.tensor_mul(fc[:], act_if[:, 4:8, :], cT_prev[:])
            c_new = work.tile([128, 4, BL], F32, tag="c")
            nc.vector.tensor_add(c_new[:], fc[:], ig[:])
            # tanh(c) ahead of sigmoid(o) in the ACT queue: c_new resolves
            # ~3/4 through the matmul burst, sigmoid(o) only at its end,
            # so this hides tanh(c) and shortens the post-burst tail to
            # sigmoid(o) -> h-mul.
            tc_b = work.tile([128, 4, BL], BF16, tag="tanhc")
            nc.scalar.activation(tc_b[:], c_new[:], AF.Tanh)
            nc.scalar.activation(act_o[:], pg_og[:, 4:8, :], AF.Sigmoid)
            h_new = hstore[:, :, lo : lo + BL]
            nc.vector.tensor_mul(h_new, act_o[:], tc_b[:])
            return h_new, c_new

        # ---- phase 2: the 20 serial LSTM steps
        if h0_zero:
            hT_prev, cT_prev = emit_step0_zero()
        else:
            hT_prev, cT_prev = h0T, c0T

        # bridge spins: phase 1A ends with the xsT-head DMA, whhT is ~2MB
        # behind on the same ring - keep the PE busy until it lands. The
        # xgT read pins these after phase-1A in the schedule.
        for _ in range(SPIN2):
            nc.tensor.matmul(
                psp[:, 0:256], spinw[:], xgT[:, 12:16, 0:XG_A],
                start=True, stop=True,
            )

        # unit queue honoring availability: chunk 0 (h steps 0-7) from
        # step 8, chunk 1 (h steps 8-15) from step 16, chunk 2 post-loop
        units = [(0, ni) for ni in range(len(N_SLICES))]
        units += [(1, ni) for ni in range(len(N_SLICES))]
        units += [(2, ni) for ni in range(len(N_SLICES) // 2)]
        avail = {0: 8, 1: 15, 2: 19}  # tail t runs after step t's h: chunk1 usable at t=15
        uq = [0]  # next unit index

        def take_units(t, n):
            for _ in range(n):
                if uq[0] >= len(units):
                    return
                ci, ni = units[uq[0]]
                if t < avail[ci]:
                    return
                uq[0] += 1
                emit_logits_unit(ci, ni)

        t0 = 1 if h0_zero else 0
        if not h0_zero:
            hT_prev, cT_prev = emit_step(0, hT_prev, cT_prev)
        for t in range(t0, T):
            hT_prev, cT_prev = emit_step(t, hT_prev, cT_prev)
            if t in (1, 2, 3, 4):
                emit_xg_block(192, R, range(4 * (t - 1), 4 * t),
                              allow_act=False)
            elif t < 8:
                # junk matmuls pinned behind this step's h (one h-dep read,
                # then wide high-MAC-duty spins) keep the PE and its HAM
                # clock busy while the cell's ACT/DVE chain runs.
                nc.tensor.matmul(
                    psp[:, 0:64], spinw[:], hT_prev[:, 0:4, :],
                    start=True, stop=True,
                )
                for _ in range(TAIL_SPIN):
                    nc.tensor.matmul(psp[:], spinw[:], spinx[:],
                                     start=True, stop=True)
            else:
                take_units(t, 3)

        # ---- phase 3: remaining logits units, dense. Chunk-2 pairs (two
        # out-DMAs each) go first; singles last so the final drain is light.
        rest = units[uq[0] :]
        rest = [u for u in rest if u[0] == 2] + [u for u in rest if u[0] != 2]
        for j, (ci, ni) in enumerate(rest):
            emit_logits_unit(ci, ni, use_act=(j % 3 == 2), act_dma=(j % 3 == 2))

    nc.compile()
    return nc


def _prep_inputs(features, captions, h0, c0, embed_w, W_ih, W_hh, b_ih, b_hh,
                 W_lin, b_lin, h0_zero):
    """Host-side layout prep (data movement only). Returns per-core in_maps."""
    bf = ml_dtypes.bfloat16
    f32 = np.float32

    features = np.asarray(features, f32)
    captions = np.asarray(captions)
    embed_w = np.asarray(embed_w, f32)
    W_ih = np.asarray(W_ih, f32)
    W_hh = np.asarray(W_hh, f32)
    b_ih = np.asarray(b_ih, f32)
    b_hh = np.asarray(b_hh, f32)
    W_lin = np.asarray(W_lin, f32)
    b_lin = np.asarray(b_lin, f32)

    # xs: [B, T, E] = [features, embed(captions[:, :T-1])]
    xs = np.empty((B, T, E), f32)
    xs[:, 0, :] = features
    xs[:, 1:, :] = embed_w[captions[:, : T - 1]]

    def to_kpm(w):  # [512, M] -> [128, KT, M] with row = k*128 + p
        return np.ascontiguousarray(w.reshape(KT, 128, w.shape[1]).transpose(1, 0, 2))

    ident = np.eye(128, dtype=bf)
    wihT = to_kpm(W_ih.T).astype(bf)
    whhT = to_kpm(W_hh.T).astype(bf)
    wlinT = to_kpm(W_lin.T).astype(bf)
    bsum = np.ascontiguousarray((b_ih + b_hh).reshape(GT, 128).T).astype(f32)
    blin = b_lin.reshape(1, V).astype(bf)

    in_maps = []
    for j in range(NC):
        sl = slice(j * BL, (j + 1) * BL)
        x = xs[sl]  # [BL, T, E]
        xsT = x.transpose(2, 1, 0).reshape(KT, 128, T * BL).transpose(1, 0, 2)
        im = {
            "ident": ident,
            "xsT": np.ascontiguousarray(xsT).astype(bf),
            "wihT": wihT,
            "whhT": whhT,
            "bsum": bsum,
            "wlinT": wlinT,
            "blin": blin,
        }
        if not h0_zero:
            h0 = np.asarray(h0, f32)
            c0 = np.asarray(c0, f32)
            h0T = h0[sl].T.reshape(KT, 128, BL).transpose(1, 0, 2)
            c0T = c0[sl].T.reshape(KT, 128, BL).transpose(1, 0, 2)
            im["h0T"] = np.ascontiguousarray(h0T).astype(bf)
            im["c0T"] = np.ascontiguousarray(c0T).astype(f32)
        in_maps.append(im)
    return in_maps


def kernel(**inputs) -> np.ndarray:
    maxlen = int(inputs.get("maxlen", T))
    assert maxlen == T, f"kernel hardcodes T={T}, got maxlen={maxlen}"
    use_blin = bool(np.any(np.asarray(inputs["b_lin"])))
    h0_zero = not (np.any(np.asarray(inputs["h0"]))
                   or np.any(np.asarray(inputs["c0"])))
    key = ("nc", use_blin, h0_zero)
    if key not in _cache:
        _cache[key] = _build_nc(use_blin, h0_zero)
    nc = _cache[key]
    in_maps = _prep_inputs(
        inputs["features"], inputs["captions"], inputs["h0"], inputs["c0"],
        inputs["embed_w"], inputs["W_ih"], inputs["W_hh"], inputs["b_ih"],
        inputs["b_hh"], inputs["W_lin"], inputs["b_lin"], h0_zero,
    )
    res = run_bass_kernel_spmd(nc, in_maps, list(range(NC)))
    # reassemble: core j rows (t*BL + b) -> full rows (t*B + j*BL + b)
    out = np.empty((T * B, V), np.float32)
    ov = out.reshape(T, NC, BL, V)
    for j in range(NC):
        ov[:, j] = res.results[j]["out"].reshape(T, BL, V).astype(np.float32)
    return out


# revision 46
# speedup vs baseline: 1.0121x; 1.0121x over previous
"""Trainium2 Bass kernel for nn_DecoderPolicyGradient (teacher-forced LSTM decoder).

Model: B=128, T=20, E=H=512, V=10000.
  xs[t] = features (t=0) | embed(captions[:, t-1])
  (h, c) = LSTM(xs[t], (h, c));  logits[t] = h @ W_lin.T + b_lin
  out = logits, time-major flattened: [T*B, V] fp32.

Sharding: pure data-parallel over batch, B/8 = 16 rows per NeuronCore, no
collectives. Per-core plan (everything "transposed": the 128-partition axis
carries hidden/gate dims and batch lives in the free dim):

  1. XgT[2048, 320] = W_ih @ xs.T + (b_ih + b_hh): one batched matmul over
     all 20 steps; the bias rides the psum->SBUF copy (alternating ACT
     bias-activation / DVE tensor_scalar_add).
  2. 20 serial LSTM steps at B=16, gatesT[2048, 16] = W_hh @ h + XgT[:, t].
     The XgT addend is folded into the gates PSUM by a tiny identity
     matmul per gate group (psum += I.T @ xgT), so ACT applies
     sigmoid/tanh directly FROM PSUM - no per-step DVE gate adds.
     h0 == c0 == 0 in this model, so step 0 skips all matmuls:
     gates_0 = XgT[:, 0] read straight from SBUF, c_1 = i*g.
  3. logits[320, 10000] = H @ W_lin.T: 3 row-chunks x 20 vocab-slices of
     512, one unit ~= 4 matmuls of N=512. Units are interleaved into the
     recurrence step tails (1/step from step 8) and the rest run densely
     after. PSUM rotates over 4 banks; copies go to DVE (ACT every 3rd
     unit in the dense phase).

HAM (PE clock gate) management: the PE defaults to 1.2GHz and only reaches
2.4GHz after ~3.4us of sustained activity; the baseline spent 58us of its
95us PE-busy time throttled. Dummy spin matmuls fill the two DMA-bound
holes (startup, and the phase-1 -> recurrence gap waiting on W_hh) to keep
the clock pinned at 2.4GHz.
"""

import sys

sys.path.insert(0, "/opt/trn_rl_repo")

from contextlib import ExitStack

import ml_dtypes
import numpy as np

import concourse.mybir as mybir
import concourse.tile as tile
from concourse import bacc
from concourse.bass_utils import run_bass_kernel_spmd

BF16 = mybir.dt.bfloat16
F32 = mybir.dt.float32
AF = mybir.ActivationFunctionType

B, T, E, H, V = 128, 20, 512, 512, 10000
NC = 8
BL = B // NC  # 16 batch rows per core
R = BL * T  # 320 output rows per core
KT = 4  # k-tiles of 128 over E/H
GT = 16  # m-tiles of 128 over 4H
VS = 512  # vocab n-slice width
M_CHUNKS = ((0, 128), (128, 128), (256, 64))  # logits m-chunks (start, rows)
N_SLICES = [(s, min(VS, V - s)) for s in range(0, V, VS)]

SPIN1 = 4  # high-duty warmup matmuls (N=448) riding the startup DMA wait
SPIN2 = 3  # bridge from phase-1A end to W_hh arrival
TAIL_SPIN = 2  # junk matmuls per spare early-step tail (h-dep pinned)
XG_A = 64  # Xg columns computed up-front (steps 0-3); rest rides step tails

_cache = {}


def _build_nc(use_blin, h0_zero):
    nc = bacc.Bacc("TRN2", target_bir_lowering=False, debug=False)

    ident_d = nc.dram_tensor("ident", [128, 128], BF16, kind="ExternalInput").ap()
    bsum_d = nc.dram_tensor("bsum", [128, GT], F32, kind="ExternalInput").ap()
    wihT_d = nc.dram_tensor("wihT", [128, KT, 4 * H], BF16, kind="ExternalInput").ap()
    xsT_d = nc.dram_tensor("xsT", [128, KT, R], BF16, kind="ExternalInput").ap()
    whhT_d = nc.dram_tensor("whhT", [128, KT, 4 * H], BF16, kind="ExternalInput").ap()
    wlinT_d = nc.dram_tensor("wlinT", [128, KT, V], BF16, kind="ExternalInput").ap()
    blin_d = nc.dram_tensor("blin", [1, V], BF16, kind="ExternalInput").ap()
    if not h0_zero:
        h0T_d = nc.dram_tensor("h0T", [128, KT, BL], BF16, kind="ExternalInput").ap()
        c0T_d = nc.dram_tensor("c0T", [128, KT, BL], F32, kind="ExternalInput").ap()
    out_d = nc.dram_tensor("out", [R, V], BF16, kind="ExternalOutput").ap()

    with tile.TileContext(nc) as tc, ExitStack() as ctx:
        const = ctx.enter_context(tc.tile_pool(name="const", bufs=1))
        work = ctx.enter_context(tc.tile_pool(name="work", bufs=2))
        stage = ctx.enter_context(tc.tile_pool(name="stage", bufs=6))
        psum_g = ctx.enter_context(tc.tile_pool(name="psum_g", bufs=1, space="PSUM"))
        psum_l = ctx.enter_context(tc.tile_pool(name="psum_l", bufs=1, space="PSUM"))

        # ---- persistent SBUF tensors
        ident = const.tile([128, 128], BF16)
        xsT = const.tile([128, KT, R], BF16)
        wihT = const.tile([128, KT, 4 * H], BF16)
        bsum = const.tile([128, GT], F32)
        whhT = const.tile([128, KT, 4 * H], BF16)
        blin = const.tile([1, V], BF16)
        ones = const.tile([1, 128], BF16)
        wlinT = const.tile([128, KT, V], BF16)
        xgT = const.tile([128, GT, R], BF16)
        hstore = const.tile([128, KT, R], BF16)
        spinw = const.tile([128, 32], BF16)
        spinx = const.tile([128, 448], BF16)
        junk = const.tile([128, 1], F32)
        junk2 = const.tile([128, 1], BF16)
        if not h0_zero:
            h0T = const.tile([128, KT, BL], BF16)
            c0T = const.tile([128, KT, BL], F32)

        # ---- input DMAs (SP HWDGE ring, FIFO). Order = priority.
        nc.sync.dma_start(ident[:], ident_d[:])
        nc.sync.dma_start(bsum[:], bsum_d[:])
        # Descriptor issue on the sync engine costs ~0.7-1.4us EACH, so
        # keep the count low and order by when the data is needed:
        # xsT-head (phase-1A) first, wihT, xsT-mid (phase-1B), whhT
        # (recurrence), xsT-tail (phase-1C in step tails).
        nc.sync.dma_start(xsT[:, :, 0:XG_A], xsT_d[:, :, 0:XG_A])
        for q in range(2):
            nc.sync.dma_start(
                wihT[:, :, q * 1024 : (q + 1) * 1024],
                wihT_d[:, :, q * 1024 : (q + 1) * 1024],
            )
        nc.sync.dma_start(xsT[:, :, XG_A:192], xsT_d[:, :, XG_A:192])
        for q in range(2):
            nc.sync.dma_start(
                whhT[:, :, q * 1024 : (q + 1) * 1024],
                whhT_d[:, :, q * 1024 : (q + 1) * 1024],
            )
        nc.sync.dma_start(xsT[:, :, 192:R], xsT_d[:, :, 192:R])
        if not h0_zero:
            nc.sync.dma_start(h0T[:], h0T_d[:])
            nc.sync.dma_start(c0T[:], c0T_d[:])
        if use_blin:
            nc.sync.dma_start(blin[:], blin_d[:])

        nc.gpsimd.memset(spinw[:], 1.0)
        nc.gpsimd.memset(spinx[:], 0.5)
        nc.gpsimd.memset(junk[:], 0.0)
        nc.gpsimd.memset(ones[:], 1.0)

        # big W_lin load: the 16 SDMA engines drain the ring roughly
        # concurrently, so FIFO order alone would let this 10 MB steal HBM
        # bandwidth from the critical startup loads. Gate each chunk behind
        # whhT's arrival via a tiny gpsimd write into the chunk's
        # destination region (WAW) that itself reads whhT (RAW).
        def load_wlin_chunk(q):
            s = q * (V // 8)
            # gate on the LAST whhT chunk (ring FIFO: done last)
            nc.gpsimd.tensor_copy(
                wlinT[0:1, 0:1, s : s + 1], whhT[0:1, 3:4, 4 * H - 1 : 4 * H]
            )
            nc.sync.dma_start(
                wlinT[:, :, s : s + V // 8], wlinT_d[:, :, s : s + V // 8]
            )

        for q in range(4):
            load_wlin_chunk(2 * q)
            load_wlin_chunk(2 * q + 1)

        # ---- ACT table preloads (sigmoid/tanh/identity) ride the DMA wait
        nc.scalar.activation(junk2[:], junk[:], AF.Sigmoid)
        nc.scalar.activation(junk2[:], junk[:], AF.Tanh)
        nc.scalar.activation(junk2[:], junk[:], AF.Identity)

        # ---- warmup spin: keep the PE busy (and the HAM clock ramping to
        # 2.4GHz) while the startup DMAs land. High-duty shape: tiny 32-col
        # LDWEIGHTS + long N=448 matmul, ~93% MAC occupancy.
        psp = psum_g.tile([32, 448], F32, tag="sp")
        for _ in range(SPIN1):
            nc.tensor.matmul(psp[:], spinw[:], spinx[:], start=True, stop=True)

        # ---- phase 1: XgT[2048, R] = W_ih @ xs.T + bsum, in column blocks.
        # Block A (cols 0:XG_A) runs up-front; blocks B/C ride the early
        # recurrence step tails as real fill work (their columns are only
        # needed from step 4 / step 12 on).
        xg_blk = [0]

        def emit_xg_block(lo, hi, ms, allow_act=True):
            w = hi - lo
            for m in ms:
                pxg = psum_l.tile([128, R], F32, tag=f"pl{xg_blk[0] % 3}",
                                  name=f"pxg{lo}_{m}")
                xg_blk[0] += 1
                for k in range(KT):
                    nc.tensor.matmul(
                        pxg[:, 0:w],
                        wihT[:, k, m * 128 : (m + 1) * 128],
                        xsT[:, k, lo:hi],
                        start=(k == 0),
                        stop=(k == KT - 1),
                    )
                if allow_act and m % 2 == 0:
                    nc.scalar.activation(
                        xgT[:, m, lo:hi], pxg[:, 0:w], AF.Identity,
                        bias=bsum[:, m : m + 1],
                    )
                else:
                    nc.vector.tensor_scalar_add(
                        xgT[:, m, lo:hi], pxg[:, 0:w], bsum[:, m : m + 1]
                    )

        emit_xg_block(0, XG_A, range(GT), allow_act=False)
        # phase-1B right behind: real work bridging toward whhT arrival
        emit_xg_block(XG_A, 192, range(GT), allow_act=False)

        # ---- logits unit emitter
        uidx = [0]

        def mm_group(pl_ap, ms, rows, s, w):
            for k in range(KT):
                nc.tensor.matmul(
                    pl_ap,
                    hstore[:, k, ms : ms + rows],
                    wlinT[:, k, s : s + w],
                    start=(k == 0),
                    stop=(k == KT - 1) and not use_blin,
                )
            if use_blin:
                nc.tensor.matmul(
                    pl_ap, ones[:, :rows], blin[:, s : s + w],
                    start=False, stop=True,
                )

        def emit_copy_dma(pl, rows_parts, dmas, use_act, act_dma=False):
            ot = stage.tile([128, VS], BF16, tag=f"o{uidx[0] % 3}")
            cp = nc.scalar.copy if use_act else nc.vector.tensor_copy
            cp(ot[:rows_parts, :], pl[:rows_parts, :])
            dma_eng = nc.scalar if act_dma else nc.sync
            for (ms, rows, s, w, p0) in dmas:
                dma_eng.dma_start(
                    out_d[ms : ms + rows, s : s + w], ot[p0 : p0 + rows, :w]
                )

        def emit_logits_unit(ci, ni, use_act=False, act_dma=False):
            ms, rows = M_CHUNKS[ci]
            pl = psum_l.tile([128, VS], F32, tag=f"pl{uidx[0] % 3}")
            if rows == 128:
                s, w = N_SLICES[ni]
                mm_group(pl[:rows, :w], ms, rows, s, w)
                emit_copy_dma(pl, 128, [(ms, rows, s, w, 0)], use_act, act_dma)
            else:
                # ci == 2: ni indexes a PAIR of n-slices packed by row-half
                pair = N_SLICES[2 * ni : 2 * ni + 2]
                dmas = []
                for half, (s, w) in enumerate(pair):
                    mm_group(pl[64 * half : 64 * half + rows, :w], ms, rows, s, w)
                    dmas.append((ms, rows, s, w, 64 * half))
                emit_copy_dma(pl, 64 * len(pair), dmas, use_act, act_dma)
            uidx[0] += 1

        # ---- step 0 cell (h0 == c0 == 0): gates = XgT[:, :, 0:16] directly
        def emit_step0_zero():
            act_if = work.tile([128, 8, BL], BF16, tag="actif")
            act_g = work.tile([128, 4, BL], BF16, tag="actg")
            act_o = work.tile([128, 4, BL], BF16, tag="acto")
            nc.scalar.activation(act_if[:], xgT[:, 0:8, 0:BL], AF.Sigmoid)
            nc.scalar.activation(act_g[:], xgT[:, 8:12, 0:BL], AF.Tanh)
            nc.scalar.activation(act_o[:], xgT[:, 12:16, 0:BL], AF.Sigmoid)
            c_new = work.tile([128, 4, BL], F32, tag="c")
            nc.vector.tensor_mul(c_new[:], act_if[:, 0:4, :], act_g[:])
            tc_b = work.tile([128, 4, BL], BF16, tag="tanhc")
            nc.scalar.activation(tc_b[:], c_new[:], AF.Tanh)
            h_new = hstore[:, :, 0:BL]
            nc.vector.tensor_mul(h_new, act_o[:], tc_b[:])
            return h_new, c_new

        def emit_step(t, hT_prev, cT_prev):
            lo = t * BL
            # two psum tiles (separate banks) so sigmoid(i,f) only waits on
            # the first half of the matmul burst, not the whole thing
            pg_if = psum_g.tile([128, 8, BL], F32, tag=f"pa{t % 2}")
            pg_og = psum_g.tile([128, 8, BL], F32, tag=f"pb{t % 2}")
            for g in range(4):
                pg = pg_if if g < 2 else pg_og
                gl = (g % 2) * 4
                # seed the group's psum with XgT via an identity matmul;
                # the W_hh matmuls then accumulate on top of it.
                nc.tensor.matmul(
                    pg[:, gl : gl + 4, :],
                    ident[:],
                    xgT[:, 4 * g : 4 * g + 4, lo : lo + BL],
                    start=True,
                    stop=False,
                    skip_group_check=True,
                )
                for mi in range(4):
                    m = g * 4 + mi
                    for k in range(KT):
                        nc.tensor.matmul(
                            pg[:, gl + mi, :],
                            whhT[:, k, m * 128 : (m + 1) * 128],
                            hT_prev[:, k, :],
                            start=False,
                            stop=(k == KT - 1),
                            skip_group_check=True,
                        )
            act_if = work.tile([128, 8, BL], BF16, tag="actif")
            act_g = work.tile([128, 4, BL], BF16, tag="actg")
            act_o = work.tile([128, 4, BL], BF16, tag="acto")
            nc.scalar.activation(act_if[:], pg_if[:], AF.Sigmoid)
            nc.scalar.activation(act_g[:], pg_og[:, 0:4, :], AF.Tanh)
            ig = work.tile([128, 4, BL], F32, tag="ig")
            fc = work.tile([128, 4, BL], F32, tag="fc")
            nc.vector.tensor_mul(ig[:], act_if[:, 0:4, :], act_g[:])
            nc.vector.tensor_mul(fc[:], act_if[:, 4:8, :], cT_prev[:])
            c_new = work.tile([128, 4, BL], F32, tag="c")
            nc.vector.tensor_add(c_new[:], fc[:], ig[:])
            # tanh(c) ahead of sigmoid(o) in the ACT queue: c_new resolves
            # ~3/4 through the matmul burst, sigmoid(o) only at its end,
            # so this hides tanh(c) and shortens the post-burst tail to
            # sigmoid(o) -> h-mul.
            tc_b = work.tile([128, 4, BL], BF16, tag="tanhc")
            nc.scalar.activation(tc_b[:], c_new[:], AF.Tanh)
            nc.scalar.activation(act_o[:], pg_og[:, 4:8, :], AF.Sigmoid)
            h_new = hstore[:, :, lo : lo + BL]
            nc.vector.tensor_mul(h_new, act_o[:], tc_b[:])
            return h_new, c_new

        # ---- phase 2: the 20 serial LSTM steps
        if h0_zero:
            hT_prev, cT_prev = emit_step0_zero()
        else:
            hT_prev, cT_prev = h0T, c0T

        # bridge spins: phase 1A ends with the xsT-head DMA, whhT is ~2MB
        # behind on the same ring - keep the PE busy until it lands. The
        # xgT read pins these after phase-1A in the schedule.
        for _ in range(SPIN2):
            nc.tensor.matmul(
                psp[:, 0:256], spinw[:], xgT[:, 12:16, 0:XG_A],
                start=True, stop=True,
            )

        # unit queue honoring availability: chunk 0 (h steps 0-7) from
        # step 8, chunk 1 (h steps 8-15) from step 16, chunk 2 post-loop
        units = [(0, ni) for ni in range(len(N_SLICES))]
        units += [(1, ni) for ni in range(len(N_SLICES))]
        units += [(2, ni) for ni in range(len(N_SLICES) // 2)]
        avail = {0: 8, 1: 16, 2: T}
        uq = [0]  # next unit index

        def take_units(t, n):
            for _ in range(n):
                if uq[0] >= len(units):
                    return
                ci, ni = units[uq[0]]
                if t < avail[ci]:
                    return
                uq[0] += 1
                emit_logits_unit(ci, ni)

        t0 = 1 if h0_zero else 0
        if not h0_zero:
            hT_prev, cT_prev = emit_step(0, hT_prev, cT_prev)
        for t in range(t0, T):
            hT_prev, cT_prev = emit_step(t, hT_prev, cT_prev)
            if t in (1, 2, 3, 4):
                emit_xg_block(192, R, range(4 * (t - 1), 4 * t),
                              allow_act=False)
            elif t < 8:
                # junk matmuls pinned behind this step's h (one h-dep read,
                # then wide high-MAC-duty spins) keep the PE and its HAM
                # clock busy while the cell's ACT/DVE chain runs.
                nc.tensor.matmul(
                    psp[:, 0:64], spinw[:], hT_prev[:, 0:4, :],
                    start=True, stop=True,
                )
                for _ in range(TAIL_SPIN):
                    nc.tensor.matmul(psp[:], spinw[:], spinx[:],
                                     start=True, stop=True)
            else:
                take_units(t, 3)

        # ---- phase 3: remaining logits units, dense. Chunk-2 pairs (two
        # out-DMAs each) go first; singles last so the final drain is light.
        rest = units[uq[0] :]
        rest = [u for u in rest if u[0] == 2] + [u for u in rest if u[0] != 2]
        for j, (ci, ni) in enumerate(rest):
            emit_logits_unit(ci, ni, use_act=(j % 3 == 2), act_dma=(j % 3 == 2))

    nc.compile()
    return nc


def _prep_inputs(features, captions, h0, c0, embed_w, W_ih, W_hh, b_ih, b_hh,
                 W_lin, b_lin, h0_zero):
    """Host-side layout prep (data movement only). Returns per-core in_maps."""
    bf = ml_dtypes.bfloat16
    f32 = np.float32

    features = np.asarray(features, f32)
    captions = np.asarray(captions)
    embed_w = np.asarray(embed_w, f32)
    W_ih = np.asarray(W_ih, f32)
    W_hh = np.asarray(W_hh, f32)
    b_ih = np.asarray(b_ih, f32)
    b_hh = np.asarray(b_hh, f32)
    W_lin = np.asarray(W_lin, f32)
    b_lin = np.asarray(b_lin, f32)

    # xs: [B, T, E] = [features, embed(captions[:, :T-1])]
    xs = np.empty((B, T, E), f32)
    xs[:, 0, :] = features
    xs[:, 1:, :] = embed_w[captions[:, : T - 1]]

    def to_kpm(w):  # [512, M] -> [128, KT, M] with row = k*128 + p
        return np.ascontiguousarray(w.reshape(KT, 128, w.shape[1]).transpose(1, 0, 2))

    ident = np.eye(128, dtype=bf)
    wihT = to_kpm(W_ih.T).astype(bf)
    whhT = to_kpm(W_hh.T).astype(bf)
    wlinT = to_kpm(W_lin.T).astype(bf)
    bsum = np.ascontiguousarray((b_ih + b_hh).reshape(GT, 128).T).astype(f32)
    blin = b_lin.reshape(1, V).astype(bf)

    in_maps = []
    for j in range(NC):
        sl = slice(j * BL, (j + 1) * BL)
        x = xs[sl]  # [BL, T, E]
        xsT = x.transpose(2, 1, 0).reshape(KT, 128, T * BL).transpose(1, 0, 2)
        im = {
            "ident": ident,
            "xsT": np.ascontiguousarray(xsT).astype(bf),
            "wihT": wihT,
            "whhT": whhT,
            "bsum": bsum,
            "wlinT": wlinT,
            "blin": blin,
        }
        if not h0_zero:
            h0 = np.asarray(h0, f32)
            c0 = np.asarray(c0, f32)
            h0T = h0[sl].T.reshape(KT, 128, BL).transpose(1, 0, 2)
            c0T = c0[sl].T.reshape(KT, 128, BL).transpose(1, 0, 2)
            im["h0T"] = np.ascontiguousarray(h0T).astype(bf)
            im["c0T"] = np.ascontiguousarray(c0T).astype(f32)
        in_maps.append(im)
    return in_maps


def kernel(**inputs) -> np.ndarray:
    maxlen = int(inputs.get("maxlen", T))
    assert maxlen == T, f"kernel hardcodes T={T}, got maxlen={maxlen}"
    use_blin = bool(np.any(np.asarray(inputs["b_lin"])))
    h0_zero = not (np.any(np.asarray(inputs["h0"]))
                   or np.any(np.asarray(inputs["c0"])))
    key = ("nc", use_blin, h0_zero)
    if key not in _cache:
        _cache[key] = _build_nc(use_blin, h0_zero)
    nc = _cache[key]
    in_maps = _prep_inputs(
        inputs["features"], inputs["captions"], inputs["h0"], inputs["c0"],
        inputs["embed_w"], inputs["W_ih"], inputs["W_hh"], inputs["b_ih"],
        inputs["b_hh"], inputs["W_lin"], inputs["b_lin"], h0_zero,
    )
    res = run_bass_kernel_spmd(nc, in_maps, list(range(NC)))
    # reassemble: core j rows (t*BL + b) -> full rows (t*B + j*BL + b)
    out = np.empty((T * B, V), np.float32)
    ov = out.reshape(T, NC, BL, V)
    for j in range(NC):
        ov[:, j] = res.results[j]["out"].reshape(T, BL, V).astype(np.float32)
    return out
